# revision 1
# baseline (speedup 1.0000x reference)
"""Bidirectional ConvLSTM encoder for Trainium2, SPMD across 8 NeuronCores.

Module: x (8,16,3,32,32) -> bidirectional ConvLSTM (7x7 'same' convs,
hid=64, PyTorch gate order i,f,g,o) -> (8,16,128,32,32) fp32.

Sharding (one SPMD program, 8 cores):
  cores 0-3: forward direction, core i handles batch samples {2i, 2i+1}
  cores 4-7: backward direction on host-time-reversed input, core i-4
             handles samples {2(i-4), 2(i-4)+1}; host reverses outputs.

Per-core algorithm, per sample s in {0,1} (ping-ponged so one sample's
elementwise tail overlaps the other's matmuls):
  - h2[s]: (128, 38, 38) SBUF tile: partitions 0-63 = zero-padded hidden
    state h_pad, 64-127 = h_pad shifted left one column, so horizontally
    adjacent 7x7 taps (ky,kx)+(ky,kx+1) contract as one K=128 matmul.
    h3[s] holds h_pad and h_pad shifted up one row, pairing the leftover
    column-6 taps vertically; the last tap (6,6) rides as 64 extra
    partitions of the input-conv xsb tile: 21 + 3 K=128 pairs + one
    K=85 merged matmul cover all 49 taps.
  - Input conv via im2col: x is zero-padded and kx-pre-shifted on host;
    one DMA per (sample, t) gathers the (ky,kx,c) tap stack into
    (126, .)/(21, .) SBUF tiles -> K=126 + K=21 matmuls.
  - Per timestep: 4 PSUM accumulation groups (channel halves [i;f]/[g;o]
    x 2 spatial halves of 512 px), 26 matmuls each; each (s, ch) pair
    owns a 2-bank (128,1024) PSUM tensor so gate activations read full
    width in one instruction.
  - Gates: ScalarE sigmoid/tanh straight from PSUM with the LSTM bias
    folded into the activation; VectorE for the cell/hidden chain; the
    partition moves compute engines cannot do (m2 shift up, h2 low copy)
    ride the ACT HWDGE ring, h3 copies and output stores the SP ring.
"""

import os
import sys
from contextlib import ExitStack

import numpy as np

for _p in ("/opt/trn_rl_repo", "/root/.axon_site/_ro/trn_rl_repo"):
    if os.path.isdir(_p) and _p not in sys.path:
        sys.path.append(_p)

import concourse.bass as bass  # noqa: E402
import concourse.mybir as mybir  # noqa: E402
import concourse.tile as tile  # noqa: E402
from concourse.bass_utils import run_bass_kernel_spmd  # noqa: E402

F32 = mybir.dt.float32
F32R = mybir.dt.float32r
AF = mybir.ActivationFunctionType

B, T, C, H, W = 8, 16, 3, 32, 32
HID = 64
K = 7
PAD = 3
PW = H + 2 * PAD          # 38
NPIX = H * W              # 1024
XFREE = 31 * PW + 31 + 1  # 1210: max flattened window index + 1

# tap pairing. h2 holds [h_pad; h_pad shifted left one col] so taps
# (ky,kx)+(ky,kx+1) contract as one K=128 matmul; h3 holds [h_pad; h_pad
# shifted up one row] so leftover column-6 taps (ky,6)+(ky+1,6) pair too.
PAIRS = [(ky, kx) for ky in range(K) for kx in (0, 2, 4)]   # 21 on h2
PAIRS2 = [(0, 6), (2, 6), (4, 6)]                           # 3 on h3
N_A = 126   # im2col rows for ky in 0..5 (6*7*3); remainder 21 for ky=6
N_B = 21 + 64  # xsb tile: 21 x-stack rows (ky=6) + 64 rows holding the
               # padded hidden state at flat offset 6*38+6 = the (6,6) tap


def build_nc(n_t=T, reps=1):
    """Raw-bass program: explicit per-engine instruction streams + semaphores.

    This walrus build accepts at most ONE semaphore wait per instruction, so
    every multi-dependency point is expressed as standalone wait_ge
    instructions on the consuming engine followed by the real op.

    Engine programs (k = 2*t + s indexes sample-steps, samples ping-pong):
      SP (sync) ring : weight DMAs, im2col x loads (prefetched 2 steps
                       ahead), h3 shifted copies, output stores
      PE             : per k: waits + 104 matmuls (4 PSUM groups of 26)
      ACT (scalar)   : initial zero-fills; per k: 4 full-width gate
                       activations from PSUM + the m2-shift / h2-low-copy
                       DMAs on its own HWDGE ring
      DVE (vector)   : per k: c *= sig(f); m2 = sig(i)*tanh(g);
                       c += m2 (after DMA shift); h = sig(o)*tanh(c)
                       written into the h2 hi half

    Each logical DMA stream has its own semaphore (+16 per transfer) with
    issuer-side reuse waits; spe +1 per finished PSUM group, sact +1 per
    activation, sdve +1 per marked DVE op.
    """
    nc = bass.Bass()
    # xp is host-pre-shifted along kx: xp[s,t,kx,c,r,j] = x_pad[s,t,c,r,j+kx]
    # so the im2col gather DMA needs only 3 AP dims (ky, kx*c, j).
    xp_d = nc.dram_tensor("xp", [2, n_t, K, C, PW, PW], F32R, kind="ExternalInput")
    wp_d = nc.dram_tensor("whh_pairs", [128, len(PAIRS) + len(PAIRS2), 256],
                          F32R, kind="ExternalInput")
    wa_d = nc.dram_tensor("wih_a", [N_A, 256], F32R, kind="ExternalInput")
    wb_d = nc.dram_tensor("wih_b", [N_B, 256], F32R, kind="ExternalInput")
    bias_d = nc.dram_tensor("bias", [128, 2], F32, kind="ExternalInput")
    zer_d = nc.dram_tensor("zer", [1, PW * PW], F32R, kind="ExternalInput")
    out_d = nc.dram_tensor("out", [2, n_t, HID, H, W], F32, kind="ExternalOutput")

    NK = 2 * n_t * reps  # reps>1: timing builds, state not re-zeroed per rep
    INIT_DVE = 2   # c-state memsets on DVE
    W_CH0 = 16 * 4   # wa, wb, bs + the ch0 half of the whh pair pack (sw)
    Z_INIT = 16 * 6  # h2/h3 zero fills + xsb h-region zero fills (ACT ring)

    wp = nc.alloc_sbuf_tensor("wp_t", [128, len(PAIRS) + len(PAIRS2), 256], F32R)
    wa = nc.alloc_sbuf_tensor("wa_t", [N_A, 256], F32R)
    wb = nc.alloc_sbuf_tensor("wb_t", [N_B, 256], F32R)
    bs = nc.alloc_sbuf_tensor("bs_t", [128, 2], F32)
    h2 = [nc.alloc_sbuf_tensor(f"h2_{s}", [128, PW, PW], F32R) for s in range(2)]
    h3 = [nc.alloc_sbuf_tensor(f"h3_{s}", [128, PW, PW], F32R) for s in range(2)]
    cst = [nc.alloc_sbuf_tensor(f"c_{s}", [128, NPIX], F32) for s in range(2)]
    sif = [nc.alloc_sbuf_tensor(f"sif_{s}", [128, NPIX], F32) for s in range(2)]
    tgo = [nc.alloc_sbuf_tensor(f"tgo_{s}", [128, NPIX], F32) for s in range(2)]
    m2t = [nc.alloc_sbuf_tensor(f"m2_{s}", [128, NPIX], F32) for s in range(2)]
    tch = [nc.alloc_sbuf_tensor(f"tch_{s}", [128, NPIX], F32) for s in range(2)]
    # double-buffered im2col x tiles per sample (reuse period = 2 steps)
    xsa = [[nc.alloc_sbuf_tensor(f"xsa_{s}{j}", [N_A, 32, PW], F32R)
            for j in range(2)] for s in range(2)]
    xsb = [[nc.alloc_sbuf_tensor(f"xsb_{s}{j}", [N_B, 32, PW], F32R)
            for j in range(2)] for s in range(2)]
    # 8 PSUM banks: sample 0 -> banks 0-3, sample 1 -> banks 4-7.
    # One (128,1024) tensor per (s, ch) spans two banks; each matmul group
    # targets a single-bank 512-column slice, activations read full width.
    ps = [[nc.alloc_psum_tensor(f"ps_{s}{ch}", [128, NPIX], F32)
           for ch in range(2)] for s in range(2)]

    # Semaphores. The CoreSim race detector requires that when a DMA's
    # increment lands on a value some instruction waits for, the *issuing*
    # engine has itself waited for the previous completion on that sem.  So
    # each logical DMA stream gets its own semaphore and the issuer waits
    # for the stream's previous transfer before reissuing (those waits are
    # always long-satisfied - streams reuse with period >= 2 sample-steps).
    sw = nc.alloc_semaphore("sw")          # weight DMAs, wait-all once
    sw2 = nc.alloc_semaphore("sw2")        # ch1 half of the whh pair pack
    sxs = [[nc.alloc_semaphore(f"sxs{s}{j}") for j in range(2)]
           for s in range(2)]              # per xs double-buffer slot
    sm2 = [nc.alloc_semaphore(f"sm2{s}") for s in range(2)]
    sh2 = [nc.alloc_semaphore(f"sh2{s}") for s in range(2)]
    sh3 = [nc.alloc_semaphore(f"sh3{s}") for s in range(2)]
    sxh = [nc.alloc_semaphore(f"sxh{s}") for s in range(2)]
    sou = [nc.alloc_semaphore(f"sou{s}") for s in range(2)]
    szr = nc.alloc_semaphore("szr")        # 4 h2/h3 zero-fill DMAs (ACT ring)
    spe = nc.alloc_semaphore("spe")        # +1 per finished PSUM group
    sact = nc.alloc_semaphore("sact")      # +1 per activation
    sdve = nc.alloc_semaphore("sdve")      # +1 per marked DVE op

    def im2col_src(s, t, ky0, n_ky):
        base = xp_d[s, t, 0, 0, ky0, 0]
        return bass.AP(tensor=base.tensor, offset=base.offset,
                       ap=[[PW, n_ky], [PW * PW, K * C], [1, XFREE]])

    def xs_flat(tile_, p0, p1):
        return tile_[p0:p1, :, :].rearrange("p a b -> p (a b)")[:, 0:XFREE]

    def n_fills(j):
        return (n_t * reps - j + 1) // 2  # fills of xs buffer slot j

    with nc.Block() as block:

        zsrc = bass.AP(tensor=zer_d[0, 0].tensor, offset=0,
                       ap=[[0, 128], [1, PW * PW]])

        @block.sync
        def _(sp):
            # order: small weights -> first x tile -> big whh pack -> rest,
            # so PE can start k=0 input-conv matmuls ~3us in and the pair
            # matmuls as soon as the whh pack lands (zeros load in parallel
            # on the ACT ring).
            for dst, srcd in ((wa[:, :], wa_d[:, :]),
                              (wb[:, :], wb_d[:, :]),
                              (bs[:, :], bias_d[:, :])):
                sp.dma_start(out=dst, in_=srcd).then_inc(sw, 16)

            def fill(s, tg):
                f, t = tg // 2, tg % n_t
                if f >= 1:
                    sp.wait_ge(sxs[s][tg % 2], 32 * f)
                sp.dma_start(out=xs_flat(xsa[s][tg % 2], 0, N_A),
                             in_=im2col_src(s, t, 0, 6)).then_inc(
                                 sxs[s][tg % 2], 16)
                sp.dma_start(out=xs_flat(xsb[s][tg % 2], 0, 21),
                             in_=im2col_src(s, t, 6, 1)).then_inc(
                                 sxs[s][tg % 2], 16)

            fill(0, 0)
            sp.dma_start(out=wp[:, :, 0:128],
                         in_=wp_d[:, :, 0:128]).then_inc(sw, 16)
            sp.dma_start(out=wp[:, :, 128:256],
                         in_=wp_d[:, :, 128:256]).then_inc(sw2, 16)
            for k in range(1, min(4, NK)):
                fill(k % 2, k // 2)
            for k in range(NK):
                s, tg = k % 2, k // 2
                t = tg % n_t
                h2s = h2[s]
                sp.wait_ge(sdve, INIT_DVE + 3 * k + 3)
                last = (tg == n_t * reps - 1)
                if not last:
                    if tg >= 1:
                        sp.wait_ge(sh3[s], 32 * tg)
                    sp.dma_start(
                        out=h3[s][0:64, PAD:PAD + H, PAD:PAD + W],
                        in_=h2s[64:128, PAD:PAD + H, PAD - 1:PAD - 1 + W],
                    ).then_inc(sh3[s], 16)
                    sp.dma_start(
                        out=h3[s][64:128, PAD - 1:PAD - 1 + H, PAD:PAD + W],
                        in_=h2s[64:128, PAD:PAD + H, PAD - 1:PAD - 1 + W],
                    ).then_inc(sh3[s], 16)
                if tg >= 1:
                    sp.wait_ge(sou[s], 16 * tg)
                sp.dma_start(
                    out=out_d[s, t, :, :, :],
                    in_=h2s[64:128, PAD:PAD + H,
                            PAD - 1:PAD - 1 + W].bitcast(F32),
                ).then_inc(sou[s], 16)
                if k + 4 < NK:
                    sp.wait_ge(spe, 4 * k + 4)
                    fill((k + 4) % 2, (k + 4) // 2)
            for s in range(2):
                for j in range(2):
                    sp.wait_ge(sxs[s][j], 32 * n_fills(j))
                sp.wait_ge(sh3[s], 32 * (n_t * reps - 1))
                sp.wait_ge(sou[s], 16 * n_t * reps)
            sp.wait_ge(sw, W_CH0)
            sp.wait_ge(sw2, 16)

        @block.tensor
        def _(pe):
            for k in range(NK):
                s, tg = k % 2, k // 2
                if k < 2:
                    pe.wait_ge(sw, W_CH0)
                    pe.wait_ge(szr, Z_INIT)
                    pe.wait_ge(sdve, INIT_DVE)
                pe.wait_ge(sxs[s][tg % 2], 32 * (tg // 2 + 1))
                if k >= 2:
                    pe.wait_ge(spe, 4 * (k - 2) + 4)
                    pe.wait_ge(sact, 4 * (k - 2) + 3)
                    pe.wait_ge(sdve, INIT_DVE + 3 * (k - 2) + 3)
                    pe.wait_ge(sh2[s], 16 * tg)
                    pe.wait_ge(sh3[s], 32 * tg)
                    pe.wait_ge(sxh[s], 16 * tg)
                    pe.wait_ge(sou[s], 16 * tg)
                xa, xb = xsa[s][tg % 2], xsb[s][tg % 2]
                h2s, h3s = h2[s], h3[s]
                for ch in range(2):
                    if k < 2 and ch == 1:
                        pe.wait_ge(sw2, 16)
                    for nh in range(2):
                        p = ps[s][ch][:, nh * 512:(nh + 1) * 512]
                        y0 = nh * 16
                        mm = 0
                        n_mm = 2 + len(PAIRS) + len(PAIRS2)

                        def domm(lhs, rhs):
                            nonlocal mm
                            inst = nc.tensor.matmul(
                                p, lhs, rhs,
                                start=(mm == 0), stop=(mm == n_mm - 1))
                            mm += 1
                            if mm == n_mm:
                                inst.then_inc(spe, 1)
                        domm(wa[:, ch * 128:(ch + 1) * 128],
                             xa[:, y0:y0 + 16, 0:32])
                        domm(wb[:, ch * 128:(ch + 1) * 128],
                             xb[:, y0:y0 + 16, 0:32])
                        for pi, (ky, kx) in enumerate(PAIRS):
                            domm(wp[:, pi, ch * 128:(ch + 1) * 128],
                                 h2s[0:128, ky + y0:ky + y0 + 16, kx:kx + 32])
                        for pi, (ky, kx) in enumerate(PAIRS2):
                            domm(wp[:, len(PAIRS) + pi, ch * 128:(ch + 1) * 128],
                                 h3s[0:128, ky + y0:ky + y0 + 16, kx:kx + 32])

        @block.scalar
        def _(act):
            for s in range(2):
                act.dma_start(
                    out=h2[s][:, :, :].rearrange("p a b -> p (a b)"),
                    in_=zsrc).then_inc(szr, 16)
                act.dma_start(
                    out=h3[s][:, :, :].rearrange("p a b -> p (a b)"),
                    in_=zsrc).then_inc(szr, 16)
                act.dma_start(
                    out=xs_flat(xsb[s][0], 21, N_B),
                    in_=bass.AP(tensor=zer_d[0, 0].tensor, offset=0,
                                ap=[[0, 64], [1, XFREE]])).then_inc(szr, 16)
            for k in range(NK):
                s, tg = k % 2, k // 2
                t = tg % n_t
                h2s = h2[s]
                if k >= 2:
                    act.wait_ge(sdve, INIT_DVE + 3 * (k - 2) + 3)
                    act.wait_ge(sact, 4 * (k - 2) + 4)
                act.wait_ge(spe, 4 * k + 2)
                nc.scalar.activation(
                    out=sif[s][:, :], in_=ps[s][0][:, :],
                    func=AF.Sigmoid, bias=bs[:, 0:1]).then_inc(sact, 1)
                act.wait_ge(spe, 4 * k + 4)
                nc.scalar.activation(
                    out=tgo[s][0:64, :], in_=ps[s][1][0:64, :],
                    func=AF.Tanh, bias=bs[0:64, 1:2]).then_inc(sact, 1)
                nc.scalar.activation(
                    out=tgo[s][64:128, :], in_=ps[s][1][64:128, :],
                    func=AF.Sigmoid, bias=bs[64:128, 1:2]).then_inc(sact, 1)
                act.wait_ge(sdve, INIT_DVE + 3 * k + 1)
                if tg >= 1:
                    act.wait_ge(sm2[s], 16 * tg)
                act.dma_start(out=m2t[s][64:128, :],
                              in_=m2t[s][0:64, :]).then_inc(sm2[s], 16)
                act.wait_ge(sdve, INIT_DVE + 3 * k + 2)
                nc.scalar.activation(
                    out=tch[s][64:128, :], in_=cst[s][64:128, :],
                    func=AF.Tanh).then_inc(sact, 1)
                if tg == n_t * reps - 1:
                    continue  # last step: no next-state consumers
                act.wait_ge(sdve, INIT_DVE + 3 * k + 3)
                if tg >= 1:
                    act.wait_ge(sh2[s], 16 * tg)
                act.dma_start(
                    out=h2s[0:64, PAD:PAD + H, PAD:PAD + W],
                    in_=h2s[64:128, PAD:PAD + H, PAD - 1:PAD - 1 + W],
                ).then_inc(sh2[s], 16)
                # h state for the merged (6,6)-tap rows of the next step's
                # xsb tile; sourced from the h2 HI half (complete left-shifted
                # padded state, synced via sdve): h_pad[c, j+234] = hi[c, j+233]
                if tg >= 1:
                    act.wait_ge(sxh[s], 16 * tg)
                act.dma_start(
                    out=xs_flat(xsb[s][(t + 1) % 2], 21, N_B),
                    in_=h2s[64:128, :, :].rearrange(
                        "p a b -> p (a b)")[:, 233:233 + XFREE],
                ).then_inc(sxh[s], 16)
            act.wait_ge(szr, Z_INIT)
            for s in range(2):
                act.wait_ge(sm2[s], 16 * n_t * reps)
                act.wait_ge(sh2[s], 16 * (n_t * reps - 1))
                act.wait_ge(sxh[s], 16 * (n_t * reps - 1))

        @block.vector
        def _(dve):
            for s in range(2):
                nc.vector.memset(cst[s][:, :], 0.0).then_inc(sdve, 1)
            for k in range(NK):
                s, tg = k % 2, k // 2
                if k < 2:
                    dve.wait_ge(sdve, INIT_DVE)
                # c = sigmoid(f) * c  (before m2 mul so m2's sdve inc
                # carries same-engine drain knowledge for the c add)
                dve.wait_ge(sact, 4 * k + 1)
                nc.vector.tensor_mul(
                    cst[s][64:128, :], cst[s][64:128, :], sif[s][64:128, :])
                dve.wait_ge(sact, 4 * k + 2)
                nc.vector.tensor_mul(
                    m2t[s][0:64, :], sif[s][0:64, :],
                    tgo[s][0:64, :]).then_inc(sdve, 1)
                dve.wait_ge(sm2[s], 16 * (tg + 1))
                dve.wait_ge(sdve, INIT_DVE + 3 * k + 1)
                nc.vector.tensor_add(
                    cst[s][64:128, :], cst[s][64:128, :],
                    m2t[s][64:128, :]).then_inc(sdve, 1)
                dve.wait_ge(sact, 4 * k + 4)
                nc.vector.tensor_mul(
                    h2[s][64:128, PAD:PAD + H, PAD - 1:PAD - 1 + W],
                    tgo[s][64:128, :].rearrange("p (a b) -> p a b", a=H),
                    tch[s][64:128, :].rearrange("p (a b) -> p a b", a=H),
                ).then_inc(sdve, 1)
    return nc

def _pack_weights(w_ih, w_hh, b):
    wp = np.empty((128, len(PAIRS) + len(PAIRS2), 256), np.float32)
    for pi, (ky, kx) in enumerate(PAIRS):
        wp[0:64, pi, :] = w_hh[:, :, ky, kx].T
        wp[64:128, pi, :] = w_hh[:, :, ky, kx + 1].T
    for pi, (ky, kx) in enumerate(PAIRS2):
        wp[0:64, len(PAIRS) + pi, :] = w_hh[:, :, ky, kx].T
        wp[64:128, len(PAIRS) + pi, :] = w_hh[:, :, ky + 1, kx].T
    wih = np.transpose(w_ih, (2, 3, 1, 0)).reshape(147, 256)  # (ky,kx,c) x oc
    wb = np.concatenate([wih[N_A:], w_hh[:, :, 6, 6].T], axis=0)  # (85, 256)
    bias = np.stack([b[0:128], b[128:256]], axis=1)
    return {
        "whh_pairs": np.ascontiguousarray(wp),
        "wih_a": np.ascontiguousarray(wih[:N_A]),
        "wih_b": np.ascontiguousarray(wb.astype(np.float32)),
        "bias": np.ascontiguousarray(bias.astype(np.float32)),
    }


_NC_CACHE = {}


def _get_nc(n_t=T):
    if n_t not in _NC_CACHE:
        _NC_CACHE[n_t] = build_nc(n_t)
    return _NC_CACHE[n_t]


def _build_in_maps(inputs):
    x = np.ascontiguousarray(np.asarray(inputs["x"], np.float32))
    packs = {
        "f": _pack_weights(np.asarray(inputs["w_ih_f"], np.float32),
                           np.asarray(inputs["w_hh_f"], np.float32),
                           np.asarray(inputs["b_f"], np.float32)),
        "b": _pack_weights(np.asarray(inputs["w_ih_b"], np.float32),
                           np.asarray(inputs["w_hh_b"], np.float32),
                           np.asarray(inputs["b_b"], np.float32)),
    }
    in_maps = []
    for core in range(8):
        d = "f" if core < 4 else "b"
        s0 = 2 * (core % 4)
        xs = x[s0:s0 + 2]
        if d == "b":
            xs = xs[:, ::-1]
        xpad = np.zeros((2, T, C, PW, PW), np.float32)
        xpad[:, :, :, PAD:PAD + H, PAD:PAD + W] = xs
        xp = np.zeros((2, T, K, C, PW, PW), np.float32)
        for kx in range(K):
            xp[:, :, kx, :, :, 0:PW - kx] = xpad[:, :, :, :, kx:PW]
        in_maps.append({"xp": xp, "zer": np.zeros((1, PW * PW), np.float32),
                        **packs[d]})
    return in_maps


def _run(inputs, trace=False, **run_kwargs):
    in_maps = _build_in_maps(inputs)
    nc = _get_nc(T)
    res = run_bass_kernel_spmd(
        nc, in_maps, core_ids=list(range(8)), trace=trace, **run_kwargs)

    out = np.empty((B, T, 2 * HID, H, W), np.float32)
    for core in range(8):
        o = res.results[core]["out"]
        s0 = 2 * (core % 4)
        if core < 4:
            out[s0:s0 + 2, :, 0:HID] = o
        else:
            out[s0:s0 + 2, :, HID:2 * HID] = o[:, ::-1]
    return out, res


def kernel(**inputs):
    out, _ = _run(inputs, trace=False)
    return out

